# revision 4
# baseline (speedup 1.0000x reference)
"""Trainium2 Bass kernel for nn_DihedralAngleLayer.

Input:  x [2_000_000, 42] f32 (14 atoms x 3 coords per row),
        mask_matrix [4, 14] f32 one-hot carbon selector.
Output: dihedral angle per row, [2_000_000] f32.

Data-parallel across 8 NeuronCores: rows are padded to 8*250_112 and
split evenly; each core streams its 42 MB slice through SBUF in
batch-major tiles (128 partitions x G rows/partition) and computes

    a = c0-c1, b = c2-c1, d = c3-c2, m = b x d
    r=a.b p=a.d det=a.m q=b.b s=b.d
    xx = p*q - r*s        # Lagrange identity for (a x b).(d x b)
    yy = -sqrt(q)*det     # scalar triple product form of v1.(na x nb)/|v1|
    out = atan2(yy, xx)   # range-reduced arctan on the Scalar engine

entirely on the Vector/Scalar engines.
"""

import numpy as np

import concourse.bacc as bacc
import concourse.bass as bass
import concourse.mybir as mybir
from concourse.bass_utils import run_bass_kernel_spmd
from concourse.tile import TileContext

AF = mybir.ActivationFunctionType
OP = mybir.AluOpType
F32 = mybir.dt.float32

PI = float(np.pi)

N_CORES = 8
TILES = [192] * 10 + [34]          # rows per partition per tile
ROWS_PER_CORE = 128 * sum(TILES)   # 250_112
B_FULL = 2_000_000

# scratch layout per row-group (period 69 floats)
PER = 69
S_A, S_B2, S_D2, S_M = 0, 3, 9, 15
P_1, P_2 = 18, 21
D_0 = 24
RT_0, R5_0 = 39, 44
T2_0 = 49
W_XX, W_YY, W_AX, W_AY, W_SQ, W_DF, W_MN, W_MX = 51, 52, 53, 54, 55, 56, 57, 58
W_RMX, W_RQ, W_E1, W_E2, W_SY, W_U, W_V, W_W2 = 59, 60, 61, 62, 63, 64, 65, 66
W_AL, W_Z = 67, 68


def _ap(base, off, dims):
    return bass.AP(
        base.tensor, base.offset + off, [list(base.ap[0])] + [list(d) for d in dims]
    )


def _emit_tile(nc, xp, scp, outp, x, y, start, G, c0, c1, c2, c3):
    v, s = nc.vector, nc.scalar

    xt = xp.tile([128, G * 42], F32, tag="x")
    sc = scp.tile([128, G * PER], F32, tag="sc")
    ot = outp.tile([128, G], F32, tag="o")

    nc.gpsimd.dma_start(
        out=xt[:],
        in_=x[start : start + 128 * G, :].rearrange("(p g) c -> p (g c)", p=128),
    )

    xa, sa = xt[:], sc[:]

    def xap(off, dims):
        return _ap(xa, off, [[42, G]] + dims)

    def sap(off, dims=()):
        return _ap(sa, off, [[PER, G]] + list(dims))

    v.tensor_tensor(sap(S_A, [[1, 3]]), xap(c0, [[1, 3]]), xap(c1, [[1, 3]]), OP.subtract)
    v.tensor_tensor(
        sap(S_B2, [[3, 2], [1, 3]]),
        xap(c2, [[0, 2], [1, 3]]),
        xap(c1, [[0, 2], [1, 3]]),
        OP.subtract,
    )
    v.tensor_tensor(
        sap(S_D2, [[3, 2], [1, 3]]),
        xap(c3, [[0, 2], [1, 3]]),
        xap(c2, [[0, 2], [1, 3]]),
        OP.subtract,
    )
    v.tensor_tensor(
        sap(P_1, [[3, 2], [1, 3]]),
        sap(S_B2 + 1, [[1, 2], [1, 3]]),
        sap(S_D2 + 2, [[-1, 2], [1, 3]]),
        OP.mult,
    )
    v.tensor_tensor(sap(S_M, [[1, 3]]), sap(P_1, [[1, 3]]), sap(P_2, [[1, 3]]), OP.subtract)
    v.tensor_tensor(
        sap(D_0, [[3, 3], [1, 3]]),
        sap(S_A, [[0, 3], [1, 3]]),
        sap(S_B2, [[6, 3], [1, 3]]),
        OP.mult,
    )
    v.tensor_tensor(
        sap(D_0 + 9, [[3, 2], [1, 3]]),
        sap(S_B2, [[0, 2], [1, 3]]),
        sap(S_B2, [[6, 2], [1, 3]]),
        OP.mult,
    )
    v.tensor_tensor(sap(RT_0, [[1, 5]]), sap(D_0, [[3, 5]]), sap(D_0 + 1, [[3, 5]]), OP.add)
    v.tensor_tensor(sap(R5_0, [[1, 5]]), sap(RT_0, [[1, 5]]), sap(D_0 + 2, [[3, 5]]), OP.add)
    v.tensor_tensor(
        sap(T2_0, [[1, 2]]), sap(R5_0 + 1, [[-1, 2]]), sap(R5_0 + 3, [[1, 2]]), OP.mult
    )
    v.tensor_tensor(sap(W_XX), sap(T2_0), sap(T2_0 + 1), OP.subtract)
    s.activation(sap(W_SQ), sap(R5_0 + 3), AF.Sqrt)
    v.tensor_tensor(sap(W_YY), sap(W_SQ), sap(R5_0 + 2), OP.mult)
    s.activation(sap(W_AX, [[1, 2]]), sap(W_XX, [[1, 2]]), AF.Abs)
    v.tensor_tensor(sap(W_DF), sap(W_AX), sap(W_AY), OP.subtract)
    v.tensor_tensor(sap(W_MN), sap(W_AX), sap(W_AY), OP.min)
    v.tensor_tensor(sap(W_MX), sap(W_AX), sap(W_AY), OP.max)
    v.reciprocal_approx_fast(sap(W_RMX), sap(W_MX))
    v.tensor_tensor(sap(W_RQ), sap(W_MN), sap(W_RMX), OP.mult)
    s.activation(sap(W_AL), sap(W_RQ), AF.Arctan)
    s.activation(sap(W_E1), sap(W_DF), AF.Sign)
    s.activation(sap(W_E2, [[1, 2]]), sap(W_XX, [[1, 2]]), AF.Sign)
    v.tensor_tensor(sap(W_U), sap(W_E1), sap(W_E2), OP.mult)
    v.tensor_tensor(sap(W_V), sap(W_AL), sap(W_U), OP.mult)
    v.scalar_tensor_tensor(sap(W_W2), sap(W_U), PI / 4, sap(W_V), OP.mult, OP.subtract)
    v.scalar_tensor_tensor(sap(W_Z), sap(W_E2), PI / 4, sap(W_W2), OP.mult, OP.add)
    v.scalar_tensor_tensor(ot[:], sap(W_Z), PI / 2, sap(W_SY), OP.subtract, OP.mult)
    nc.gpsimd.dma_start(
        out=y[start : start + 128 * G].rearrange("(p g) -> p g", p=128),
        in_=ot[:],
    )


def build_kernel(atoms):
    c0, c1, c2, c3 = (3 * int(a) for a in atoms)
    nc = bacc.Bacc("TRN2", target_bir_lowering=False, debug=False)
    x = nc.dram_tensor("x", [ROWS_PER_CORE, 42], F32, kind="ExternalInput")
    y = nc.dram_tensor("y", [ROWS_PER_CORE], F32, kind="ExternalOutput")
    with TileContext(nc) as tc:
        with (
            tc.tile_pool(name="xp", bufs=2) as xp,
            tc.tile_pool(name="scp", bufs=2) as scp,
            tc.tile_pool(name="outp", bufs=2) as outp,
        ):
            start = 0
            for G in TILES:
                _emit_tile(nc, xp, scp, outp, x, y, start, G, c0, c1, c2, c3)
                start += 128 * G
    nc.finalize()
    return nc


_CACHE = {}


def _get_nc(atoms):
    key = tuple(int(a) for a in atoms)
    if key not in _CACHE:
        _CACHE[key] = build_kernel(key)
    return _CACHE[key]


def run(x, atoms=(0, 4, 7, 11), **spmd_kwargs):
    """x: [B, 42] f32. Returns (y [B] f32, BassKernelResults)."""
    x = np.ascontiguousarray(np.asarray(x, dtype=np.float32))
    B = x.shape[0]
    total = N_CORES * ROWS_PER_CORE
    if B < total:
        # pad with replicated leading rows (valid, non-degenerate data)
        x = np.concatenate([x, x[: total - B]], axis=0)
    nc = _get_nc(atoms)
    shards = x.reshape(N_CORES, ROWS_PER_CORE, 42)
    in_maps = [{"x": shards[i]} for i in range(N_CORES)]
    res = run_bass_kernel_spmd(nc, in_maps, core_ids=list(range(N_CORES)), **spmd_kwargs)
    y = np.concatenate([r["y"] for r in res.results])[:B]
    return np.asarray(y, dtype=np.float32), res


def kernel(x, mask_matrix):
    mask = np.asarray(mask_matrix)
    atoms = tuple(int(i) for i in np.argmax(mask, axis=1))
    y, _ = run(x, atoms=atoms)
    return y
